# revision 39
# baseline (speedup 1.0000x reference)
"""Trainium2 Bass kernel: per-class precision/recall sums via fp8 gram matmuls.

Computes, for pred/gt 0-1 indicator tensors of shape [N, C]:
    intersection = sum_n pred*gt   [C]
    pred_sum     = sum_n pred      [C]
    gt_sum       = sum_n gt        [C]
    precisions   = (intersection + EPS) / (pred_sum + EPS)
    recalls      = (intersection + EPS) / (gt_sum + EPS)

Sharding: rows split across 8 NeuronCores. Values are 0/1 indicators, so
the host re-encodes them losslessly as fp8_e4m3 (1 byte/elem) — 4x less
HBM traffic than the f32 originals (16.8 MiB/core, ~40 us at the ~420 GB/s
per-core DMA rate, vs 182 us for f32).

Host staging per core: x[tile=16, p=128, free=8224] fp8, where free is 32
groups of 257 cols: [pred(128) | ones(1) | gt(128)]. A group's 128 cols
are (class c, subrow r) pairs, col = c*8+r; its 128 partitions each hold
a distinct row, so one group covers 1024 rows.

Device pipeline per core:
  - Input DMAs ride the two HWDGE queues (sync + scalar engines) — the
    gpsimd SWDGE path costs ~9 us of descriptor-generation ramp-up.
    Last tile lands in 4 quarters so compute can chase the stream's tail.
    All 16 SBUF slots are resident — no recycling.
  - TensorE, per group: matmul lhsT = pred cols (128-wide), rhs =
    [ones | gt] (N=129), accumulating into ps_gram[128, 129]:
      col 0   = per-(c,r) pred sums   (weights x ones column)
      diag of cols 1..129 = per-(c,r) intersections
  - gt sums are split between the two engines that have slack: VectorE
    strided-reduces groups 0..17 of each tile (143 ns/group) into gtacc;
    TensorE sums groups 18..31 with ones-weight N=256 matmuls (57
    ns/group) into ps_sum2[1, 256]. The per-tile sum matmuls run
    back-to-back so the ones weights load once per tile and the next
    gram's 128-col weight load hides under their streaming. The last
    tile uses the quarter-aligned 16/16 split so both engines can chase
    its quarter DMAs.
  - No device epilogue: DVE copies ps_gram/ps_sum2 to SBUF (DMA cannot
    read PSUM) and the partial tensors go to HBM raw — o1[128, 129],
    o2[1, 256], o3 = gtacc[128, segs*16]. The host extracts the diag,
    folds the span/subrow/partition axes, sums the 8 cores' partials
    (exact integers in f64), and applies the epsilon math.

Measured on HW: 66.9 / 70.4 us across two identical runs (+-3.5 us
run-to-run jitter; f32 baseline: 182.5 us). Breakdown: ~9 us fixed
NEFF/runtime prologue before the first input byte, ~40 us DMA stream at
the ~422 GB/s per-core ceiling, ~10 us engine trail (PE runs gapless;
sum matmuls cost ~155 ns each, cause unidentified), ~3 us postlogue.
Failed experiments, for the record: 8x64 tiles (16 KB lines, same DMA
rate, worse fill: 74.8 us), per-tile partition-split DMAs (260 GB/s:
87 us), per-tile column-half DMAs (~300 GB/s: 78 us), fp8 DoubleRow
(LDW-bound), alternating dual psum banks for the sum matmuls + both
tail tiles quartered (71.6 us — small tail descriptors stretch the
stream and the bank alternation did not fix the sum-MM pace).
"""

from contextlib import ExitStack

import numpy as np

N_CORES = 8
N_ROWS, C = 4194304, 16
ROWS_PER_CORE = N_ROWS // N_CORES  # 524288
EPS = np.float32(1e-6)

P = 128              # partitions; also pred/gt cols per group (16 classes x 8 subrows)
R_SUB = 8            # subrows folded into a group's column block
GCOLS = 2 * P + 1    # 257: [pred(128) | ones(1) | gt(128)]
GROUPS_PER_TILE = 32
N_TILES = ROWS_PER_CORE // (P * R_SUB * GROUPS_PER_TILE)  # 16
TILE_FREE = GROUPS_PER_TILE * GCOLS  # 8224
N_QUARTERS = 4       # last tile split so PE/DVE finish right after the stream
SPAN = 2             # groups per TensorE sum-matmul (2 x 128 = 256 psum cols)

_F8_ONE = np.uint8(0x38)  # 1.0 in float8_e4m3

_CACHE = {}
LAST_RUN = None  # BassKernelResults of the most recent run (for test harness)


def _build_nc(n_tiles=N_TILES, groups_per_tile=GROUPS_PER_TILE):
    import concourse.bass as bass
    import concourse.mybir as mybir

    f32 = mybir.dt.float32
    fp8 = mybir.dt.float8e4

    tile_free = groups_per_tile * GCOLS
    # DVE reduces groups [0, g_half); TensorE sums the rest. 18/32 matches
    # the measured engine rates (DVE 143 ns/group, PE 57 ns/group on top of
    # its grams). The last tile uses the quarter-aligned 16/32 split.
    g_half_main = max(SPAN, (groups_per_tile * 18 // 32) // SPAN * SPAN)
    g_half_last = groups_per_tile // 2
    # gpsimd takes GP_GROUPS groups per tile off TensorE's share with a
    # pairwise-add chain (it is otherwise idle); needs >= 4 spare groups
    gp_n = 4 if groups_per_tile - g_half_main >= 2 * SPAN + 4 else 0
    pe0_main = g_half_main + gp_n          # PE sum spans start here
    pe0_last = g_half_last + gp_n
    gq = groups_per_tile // N_QUARTERS     # groups per quarter (last tile)
    n_segs = n_tiles + 1                   # gtacc segments (last tile -> 2)
    n_groups = n_tiles * groups_per_tile
    n_sums = ((n_tiles - 1) * (groups_per_tile - pe0_main)
              + (groups_per_tile - pe0_last)) // SPAN
    n_gp_ops = (3 + (n_tiles - 1) * 4) if gp_n else 0

    nc = bass.Bass()
    x_d = nc.dram_tensor("x", [n_tiles, P, tile_free], fp8, kind="ExternalInput")
    o1_d = nc.dram_tensor("o1", [P, P + 1], f32, kind="ExternalOutput")
    o2_d = nc.dram_tensor("o2", [1, SPAN * P], f32, kind="ExternalOutput")
    o3_d = nc.dram_tensor("o3", [P, n_segs * C], f32, kind="ExternalOutput")
    o4_d = nc.dram_tensor("o4", [P, P], f32, kind="ExternalOutput")

    ctx = ExitStack()
    with ctx:
        gtacc = ctx.enter_context(nc.sbuf_tensor("gtacc", [P, n_segs * C], f32))
        gbuf = ctx.enter_context(nc.sbuf_tensor("gbuf", [P, P + 1], f32))
        s2buf = ctx.enter_context(nc.sbuf_tensor("s2buf", [1, SPAN * P], f32))
        tmpA = ctx.enter_context(nc.sbuf_tensor("tmpA", [P, P], f32))
        tmpB = ctx.enter_context(nc.sbuf_tensor("tmpB", [P, P], f32))
        accs = [ctx.enter_context(nc.sbuf_tensor(f"acc{i}", [P, P], f32))
                for i in range(2)]
        slots = [
            ctx.enter_context(nc.sbuf_tensor(f"xt{t}", [P, tile_free], fp8))
            for t in range(n_tiles)
        ]

        ps_gram = ctx.enter_context(nc.psum_tensor([P, P + 1], f32))
        ps_sum2 = ctx.enter_context(nc.psum_tensor([1, SPAN * P], f32))

        tsems = [
            ctx.enter_context(nc.semaphore(name=f"t{t}"))
            for t in range(n_tiles - 1)
        ]
        qsems = [
            ctx.enter_context(nc.semaphore(name=f"q{k}"))
            for k in range(N_QUARTERS)
        ]
        pe_sem = ctx.enter_context(nc.semaphore(name="pe"))
        g_sem = ctx.enter_context(nc.semaphore(name="gpadd"))
        v_sem = ctx.enter_context(nc.semaphore(name="vself"))
        out_sem = ctx.enter_context(nc.semaphore(name="outd"))
        block = ctx.enter_context(nc.Block(no_gpsimd_drain=True))

        def grouped(slot):
            return slot[:, :].rearrange("p (f col) -> p f col",
                                        f=groups_per_tile)

        def gt_reduce_view(slot, f0, f1):
            # [p, c, f, r] view of the gt sections of groups [f0, f1)
            v = grouped(slot)[:, f0:f1, P + 1:GCOLS]
            return v.rearrange("p f (c r) -> p c f r", r=R_SUB)

        last = n_tiles - 1
        qf = tile_free // N_QUARTERS

        # whole-tile DMAs, even/odd tiles across the two HWDGE queues:
        # measured fastest (~422 GB/s). Splitting tiles by partition range
        # (260 GB/s) or column halves (~300 GB/s) both collapse the rate —
        # keep full [128, 8224] descriptors.
        @block.sync
        def _(sync):
            for t in range(0, n_tiles - 1, 2):
                sync.dma_start(slots[t][:], x_d[t]).then_inc(tsems[t], 16)
            # partial outputs, once DVE finished its reduces + psum copies
            sync.wait_ge(v_sem, n_segs + 2)
            sync.dma_start(o1_d[:, :], gbuf[:]).then_inc(out_sem, 16)
            sync.dma_start(o2_d[:, :], s2buf[:]).then_inc(out_sem, 16)
            sync.dma_start(o3_d[:, :], gtacc[:]).then_inc(out_sem, 16)
            if gp_n:
                sync.wait_ge(g_sem, n_gp_ops)
                sync.dma_start(o4_d[:, :], accs[0][:]).then_inc(out_sem, 16)
                sync.wait_ge(out_sem, 64)
            else:
                sync.wait_ge(out_sem, 48)

        @block.scalar
        def _(scalar):
            for t in range(1, n_tiles - 1, 2):
                scalar.dma_start(slots[t][:], x_d[t]).then_inc(tsems[t], 16)
            for k in range(N_QUARTERS):
                scalar.dma_start(
                    slots[last][:, k * qf:(k + 1) * qf],
                    x_d[last][:, k * qf:(k + 1) * qf],
                ).then_inc(qsems[k], 16)

        if gp_n:
            @block.gpsimd
            def _(gpsimd):
                def gtsec(t, g):
                    base = g * GCOLS
                    return slots[t][:, base + P + 1:base + GCOLS]

                nop = [0]

                def gadd(out, a, b):
                    # fully serialized chain: Q7 cores race back-to-back ops
                    if nop[0]:
                        gpsimd.wait_ge(g_sem, nop[0])
                    gpsimd.tensor_tensor(
                        out[:, :], a, b,
                        op=mybir.AluOpType.add).then_inc(g_sem, 1)
                    nop[0] += 1

                for t in range(n_tiles):
                    if t < n_tiles - 1:
                        f0 = pe0_main - gp_n
                        gpsimd.wait_ge(tsems[t], 16)
                    else:
                        f0 = pe0_last - gp_n
                        gpsimd.wait_ge(qsems[2], 16)  # gp groups are in q2
                    gadd(tmpA, gtsec(t, f0), gtsec(t, f0 + 1))
                    gadd(tmpB, gtsec(t, f0 + 2), gtsec(t, f0 + 3))
                    if t == 0:
                        gadd(accs[0], tmpA[:, :], tmpB[:, :])
                    else:
                        gadd(accs[1], accs[0][:, :], tmpA[:, :])
                        gadd(accs[0], accs[1][:, :], tmpB[:, :])
                assert nop[0] == n_gp_ops

        @block.vector
        def _(vector):
            for t in range(n_tiles - 1):
                vector.wait_ge(tsems[t], 16)
                vector.tensor_reduce(
                    gtacc[:, t * C:(t + 1) * C],
                    gt_reduce_view(slots[t], 0, g_half_main),
                    axis=mybir.AxisListType.XY,
                    op=mybir.AluOpType.add).then_inc(v_sem, 1)
            # last tile: its DVE half arrives as quarters 0 and 1
            for k in range(2):
                vector.wait_ge(qsems[k], 16)
                seg = n_tiles - 1 + k
                vector.tensor_reduce(
                    gtacc[:, seg * C:(seg + 1) * C],
                    gt_reduce_view(slots[last], k * gq, (k + 1) * gq),
                    axis=mybir.AxisListType.XY,
                    op=mybir.AluOpType.add).then_inc(v_sem, 1)
            # copy the psum partials to SBUF so DMA can ship them
            vector.wait_ge(pe_sem, 1)
            vector.tensor_scalar_mul(gbuf[:, :], ps_gram[:, :],
                                     1.0).then_inc(v_sem, 1)
            vector.tensor_scalar_mul(s2buf[:, :], ps_sum2[:, :],
                                     1.0).then_inc(v_sem, 1)

        @block.tensor
        def _(tensor):
            mm = [0, 0]  # gram count, sum count

            def gram(t, g):
                base = g * GCOLS
                inst = nc.tensor.matmul(
                    ps_gram[:, :],
                    slots[t][:, base:base + P],
                    slots[t][:, base + P:base + GCOLS],
                    start=(mm[0] == 0), stop=(mm[0] == n_groups - 1))
                mm[0] += 1
                return inst

            def gtsum(t, f0):
                inst = nc.tensor.matmul(
                    ps_sum2[:, :],
                    slots[t][:, P:P + 1],  # group 0's staged ones column
                    grouped(slots[t])[:, f0:f0 + SPAN, P + 1:GCOLS],
                    start=(mm[1] == 0), stop=(mm[1] == n_sums - 1))
                mm[1] += 1
                return inst

            for t in range(n_tiles - 1):
                tensor.wait_ge(tsems[t], 16)
                # sum matmuls back-to-back: one ones-LDW per tile, and the
                # first gram's 128-col LDW pulls ahead under their streaming
                for f0 in range(pe0_main, groups_per_tile, SPAN):
                    gtsum(t, f0)
                for g in range(groups_per_tile):
                    gram(t, g)
            # last tile: chase the quarter DMAs; sum spans are
            # quarter-aligned (issue each span after its last quarter)
            for k in range(N_QUARTERS):
                tensor.wait_ge(qsems[k], 16)
                for f0 in range(pe0_last, groups_per_tile, SPAN):
                    if k * gq <= f0 + SPAN - 1 < (k + 1) * gq:
                        gtsum(last, f0)
                for g in range(k * gq, (k + 1) * gq):
                    final = gram(last, g)
            # the final main-loop instruction carries the completion inc
            final.then_inc(pe_sem, 1)
            assert mm[0] == n_groups and mm[1] == n_sums

    return nc


def _pack_core(pred_c, gt_c, n_tiles=N_TILES, groups_per_tile=GROUPS_PER_TILE):
    """Stage one core's rows as [n_tiles, P, tile_free] fp8 bytes (uint8)."""
    shp = (n_tiles, P, groups_per_tile, R_SUB, C)
    pc = np.asarray(pred_c).reshape(shp)
    gc = np.asarray(gt_c).reshape(shp)
    X = np.empty((n_tiles, P, groups_per_tile, GCOLS), np.uint8)
    # cols are (c, r) pairs, col = c*R_SUB + r -> transpose r and c
    X[..., 0:P] = (pc.transpose(0, 1, 2, 4, 3) != 0).reshape(
        n_tiles, P, groups_per_tile, P) * _F8_ONE
    X[..., P] = _F8_ONE
    X[..., P + 1:GCOLS] = (gc.transpose(0, 1, 2, 4, 3) != 0).reshape(
        n_tiles, P, groups_per_tile, P) * _F8_ONE
    return X.reshape(n_tiles, P, groups_per_tile * GCOLS)


def _unpack_out(o1, o2, o3, o4):
    """Fold one core's raw partials (f64) -> (I, pred_sum, gt_sum), each [C]."""
    diag = o1[np.arange(P), 1 + np.arange(P)]        # I by (c, r)
    inter = diag.reshape(C, R_SUB).sum(axis=1)
    pred_sum = o1[:, 0].reshape(C, R_SUB).sum(axis=1)
    gt_pe = o2.reshape(SPAN, P).sum(axis=0).reshape(C, R_SUB).sum(axis=1)
    gt_dve = o3.reshape(P, -1, C).sum(axis=(0, 1))
    gt_gp = o4.sum(axis=0).reshape(C, R_SUB).sum(axis=1)
    return inter, pred_sum, gt_dve + gt_pe + gt_gp


def _get_nc():
    if "nc" not in _CACHE:
        _CACHE["nc"] = _build_nc()
    return _CACHE["nc"]


def kernel(pred, gt, **run_kwargs):
    global LAST_RUN
    import ml_dtypes
    from concourse.bass_utils import run_bass_kernel_spmd

    pred = np.asarray(pred)
    gt = np.asarray(gt)
    assert pred.shape == (N_ROWS, C) and gt.shape == (N_ROWS, C)

    in_maps = []
    for i in range(N_CORES):
        sl = slice(i * ROWS_PER_CORE, (i + 1) * ROWS_PER_CORE)
        X = _pack_core(pred[sl], gt[sl])
        in_maps.append({"x": X.view(ml_dtypes.float8_e4m3)})

    nc = _get_nc()
    br = run_bass_kernel_spmd(nc, in_maps, core_ids=list(range(N_CORES)),
                              **run_kwargs)
    LAST_RUN = br

    inter = np.zeros(C)
    pred_sum = np.zeros(C)
    gt_sum = np.zeros(C)
    for r in br.results:
        i_, p_, g_ = _unpack_out(r["o1"].astype(np.float64),
                                 r["o2"].astype(np.float64).reshape(-1),
                                 r["o3"].astype(np.float64),
                                 r["o4"].astype(np.float64))
        inter += i_
        pred_sum += p_
        gt_sum += g_
    inter = inter.astype(np.float32)
    pred_sum = pred_sum.astype(np.float32)
    gt_sum = gt_sum.astype(np.float32)

    recalls = (inter + EPS) / (gt_sum + EPS)
    precisions = (inter + EPS) / (pred_sum + EPS)
    return (precisions, recalls, inter, gt_sum, pred_sum)


# revision 41
# speedup vs baseline: 1.0997x; 1.0997x over previous
"""Trainium2 Bass kernel: per-class precision/recall sums via fp8 gram matmuls.

Computes, for pred/gt 0-1 indicator tensors of shape [N, C]:
    intersection = sum_n pred*gt   [C]
    pred_sum     = sum_n pred      [C]
    gt_sum       = sum_n gt        [C]
    precisions   = (intersection + EPS) / (pred_sum + EPS)
    recalls      = (intersection + EPS) / (gt_sum + EPS)

Sharding: rows split across 8 NeuronCores. Values are 0/1 indicators, so
the host re-encodes them losslessly as fp8_e4m3 (1 byte/elem) — 4x less
HBM traffic than the f32 originals (16.8 MiB/core, ~40 us at the ~420 GB/s
per-core DMA rate, vs 182 us for f32).

Host staging per core: x[tile=16, p=128, free=8224] fp8, where free is 32
groups of 257 cols: [pred(128) | ones(1) | gt(128)]. A group's 128 cols
are (class c, subrow r) pairs, col = c*8+r; its 128 partitions each hold
a distinct row, so one group covers 1024 rows.

Device pipeline per core:
  - Input DMAs ride the two HWDGE queues (sync + scalar engines) — the
    gpsimd SWDGE path costs ~9 us of descriptor-generation ramp-up.
    Last tile lands in 4 quarters so compute can chase the stream's tail.
    All 16 SBUF slots are resident — no recycling.
  - TensorE, per group: matmul lhsT = pred cols (128-wide), rhs =
    [ones | gt] (N=129), accumulating into ps_gram[128, 129]:
      col 0   = per-(c,r) pred sums   (weights x ones column)
      diag of cols 1..129 = per-(c,r) intersections
  - gt sums are split between the two engines that have slack: VectorE
    strided-reduces groups 0..17 of each tile (143 ns/group) into gtacc;
    TensorE sums groups 18..31 with ones-weight N=256 matmuls (57
    ns/group) into ps_sum2[1, 256]. The per-tile sum matmuls run
    back-to-back so the ones weights load once per tile and the next
    gram's 128-col weight load hides under their streaming. The last
    tile uses the quarter-aligned 16/16 split so both engines can chase
    its quarter DMAs.
  - No device epilogue: DVE copies ps_gram/ps_sum2 to SBUF (DMA cannot
    read PSUM) and the partial tensors go to HBM raw — o1[128, 129],
    o2[1, 256], o3 = gtacc[128, segs*16]. The host extracts the diag,
    folds the span/subrow/partition axes, sums the 8 cores' partials
    (exact integers in f64), and applies the epsilon math.

Measured on HW: 66.9 / 70.4 us across two identical runs (+-3.5 us
run-to-run jitter; f32 baseline: 182.5 us). Breakdown: ~9 us fixed
NEFF/runtime prologue before the first input byte, ~40 us DMA stream at
the ~422 GB/s per-core ceiling, ~10 us engine trail (PE runs gapless;
sum matmuls cost ~155 ns each, cause unidentified), ~3 us postlogue.
Failed experiments, for the record: 8x64 tiles (16 KB lines, same DMA
rate, worse fill: 74.8 us), per-tile partition-split DMAs (260 GB/s:
87 us), per-tile column-half DMAs (~300 GB/s: 78 us), fp8 DoubleRow
(LDW-bound), alternating dual psum banks for the sum matmuls + both
tail tiles quartered (71.6 us — small tail descriptors stretch the
stream and the bank alternation did not fix the sum-MM pace).
"""

from contextlib import ExitStack

import numpy as np

N_CORES = 8
N_ROWS, C = 4194304, 16
ROWS_PER_CORE = N_ROWS // N_CORES  # 524288
EPS = np.float32(1e-6)

P = 128              # partitions; also pred/gt cols per group (16 classes x 8 subrows)
R_SUB = 8            # subrows folded into a group's column block
GCOLS = 2 * P + 1    # 257: [pred(128) | ones(1) | gt(128)]
GROUPS_PER_TILE = 32
N_TILES = ROWS_PER_CORE // (P * R_SUB * GROUPS_PER_TILE)  # 16
TILE_FREE = GROUPS_PER_TILE * GCOLS  # 8224
N_QUARTERS = 4       # last tile split so PE/DVE finish right after the stream
SPAN = 2             # groups per TensorE sum-matmul (2 x 128 = 256 psum cols)

_F8_ONE = np.uint8(0x38)  # 1.0 in float8_e4m3

_CACHE = {}
LAST_RUN = None  # BassKernelResults of the most recent run (for test harness)


def _build_nc(n_tiles=N_TILES, groups_per_tile=GROUPS_PER_TILE):
    import concourse.bass as bass
    import concourse.mybir as mybir

    f32 = mybir.dt.float32
    fp8 = mybir.dt.float8e4

    tile_free = groups_per_tile * GCOLS
    # DVE reduces groups [0, g_half); TensorE sums the rest. 18/32 matches
    # the measured engine rates (DVE 143 ns/group, PE 57 ns/group on top of
    # its grams). The last tile uses the quarter-aligned 16/32 split.
    g_half_main = max(SPAN, (groups_per_tile * 18 // 32) // SPAN * SPAN)
    g_half_last = groups_per_tile // 2
    gq = groups_per_tile // N_QUARTERS     # groups per quarter (last tile)
    n_segs = n_tiles + 1                   # gtacc segments (last tile -> 2)
    n_groups = n_tiles * groups_per_tile
    n_sums = ((n_tiles - 1) * (groups_per_tile - g_half_main)
              + (groups_per_tile - g_half_last)) // SPAN

    nc = bass.Bass()
    x_d = nc.dram_tensor("x", [n_tiles, P, tile_free], fp8, kind="ExternalInput")
    o1_d = nc.dram_tensor("o1", [P, P + 1], f32, kind="ExternalOutput")
    o2_d = nc.dram_tensor("o2", [1, SPAN * P], f32, kind="ExternalOutput")
    o3_d = nc.dram_tensor("o3", [P, n_segs * C], f32, kind="ExternalOutput")

    ctx = ExitStack()
    with ctx:
        gtacc = ctx.enter_context(nc.sbuf_tensor("gtacc", [P, n_segs * C], f32))
        gbuf = ctx.enter_context(nc.sbuf_tensor("gbuf", [P, P + 1], f32))
        s2buf = ctx.enter_context(nc.sbuf_tensor("s2buf", [1, SPAN * P], f32))
        ones8 = ctx.enter_context(nc.sbuf_tensor("ones8", [P, P], fp8))
        slots = [
            ctx.enter_context(nc.sbuf_tensor(f"xt{t}", [P, tile_free], fp8))
            for t in range(n_tiles)
        ]

        ps_gram = ctx.enter_context(nc.psum_tensor([P, P + 1], f32))
        # M=128 all-ones weights: out rows are identical col sums, but the
        # matmul is shape-identical to the grams (M=1 outs paced ~50 ns/MM
        # slower); host reads row 0
        ps_sum2 = ctx.enter_context(nc.psum_tensor([P, SPAN * P], f32))

        tsems = [
            ctx.enter_context(nc.semaphore(name=f"t{t}"))
            for t in range(n_tiles - 1)
        ]
        qsems = [
            ctx.enter_context(nc.semaphore(name=f"q{k}"))
            for k in range(N_QUARTERS)
        ]
        pe_sem = ctx.enter_context(nc.semaphore(name="pe"))
        w_sem = ctx.enter_context(nc.semaphore(name="wready"))
        v_sem = ctx.enter_context(nc.semaphore(name="vself"))
        out_sem = ctx.enter_context(nc.semaphore(name="outd"))
        block = ctx.enter_context(nc.Block(no_gpsimd_drain=True))

        def grouped(slot):
            return slot[:, :].rearrange("p (f col) -> p f col",
                                        f=groups_per_tile)

        def gt_reduce_view(slot, f0, f1):
            # [p, c, f, r] view of the gt sections of groups [f0, f1)
            v = grouped(slot)[:, f0:f1, P + 1:GCOLS]
            return v.rearrange("p f (c r) -> p c f r", r=R_SUB)

        last = n_tiles - 1
        qf = tile_free // N_QUARTERS

        # whole-tile DMAs, even/odd tiles across the two HWDGE queues:
        # measured fastest (~422 GB/s). Splitting tiles by partition range
        # (260 GB/s) or column halves (~300 GB/s) both collapse the rate —
        # keep full [128, 8224] descriptors.
        @block.sync
        def _(sync):
            for t in range(0, n_tiles - 1, 2):
                sync.dma_start(slots[t][:], x_d[t]).then_inc(tsems[t], 16)
            # partial outputs, once DVE finished its reduces + psum copies
            sync.wait_ge(v_sem, n_segs + 2)
            sync.dma_start(o1_d[:, :], gbuf[:]).then_inc(out_sem, 16)
            sync.dma_start(o2_d[:, :], s2buf[:]).then_inc(out_sem, 16)
            sync.dma_start(o3_d[:, :], gtacc[:]).then_inc(out_sem, 16)
            sync.wait_ge(out_sem, 48)

        @block.scalar
        def _(scalar):
            for t in range(1, n_tiles - 1, 2):
                scalar.dma_start(slots[t][:], x_d[t]).then_inc(tsems[t], 16)
            for k in range(N_QUARTERS):
                scalar.dma_start(
                    slots[last][:, k * qf:(k + 1) * qf],
                    x_d[last][:, k * qf:(k + 1) * qf],
                ).then_inc(qsems[k], 16)

        @block.vector
        def _(vector):
            vector.memset(ones8[:], 1.0).then_inc(w_sem, 1)
            for t in range(n_tiles - 1):
                vector.wait_ge(tsems[t], 16)
                vector.tensor_reduce(
                    gtacc[:, t * C:(t + 1) * C],
                    gt_reduce_view(slots[t], 0, g_half_main),
                    axis=mybir.AxisListType.XY,
                    op=mybir.AluOpType.add).then_inc(v_sem, 1)
            # last tile: its DVE half arrives as quarters 0 and 1
            for k in range(2):
                vector.wait_ge(qsems[k], 16)
                seg = n_tiles - 1 + k
                vector.tensor_reduce(
                    gtacc[:, seg * C:(seg + 1) * C],
                    gt_reduce_view(slots[last], k * gq, (k + 1) * gq),
                    axis=mybir.AxisListType.XY,
                    op=mybir.AluOpType.add).then_inc(v_sem, 1)
            # copy the psum partials to SBUF so DMA can ship them
            vector.wait_ge(pe_sem, 1)
            vector.tensor_scalar_mul(gbuf[:, :], ps_gram[:, :],
                                     1.0).then_inc(v_sem, 1)
            vector.tensor_scalar_mul(s2buf[:, :], ps_sum2[0:1, :],
                                     1.0).then_inc(v_sem, 1)

        @block.tensor
        def _(tensor):
            mm = [0, 0]  # gram count, sum count

            def gram(t, g):
                base = g * GCOLS
                inst = nc.tensor.matmul(
                    ps_gram[:, :],
                    slots[t][:, base:base + P],
                    slots[t][:, base + P:base + GCOLS],
                    start=(mm[0] == 0), stop=(mm[0] == n_groups - 1))
                mm[0] += 1
                return inst

            def gtsum(t, f0):
                inst = nc.tensor.matmul(
                    ps_sum2[:, :],
                    ones8[:, :],
                    grouped(slots[t])[:, f0:f0 + SPAN, P + 1:GCOLS],
                    start=(mm[1] == 0), stop=(mm[1] == n_sums - 1))
                mm[1] += 1
                return inst

            tensor.wait_ge(w_sem, 1)  # ones8 ready (lands long before data)
            for t in range(n_tiles - 1):
                tensor.wait_ge(tsems[t], 16)
                # sum matmuls back-to-back: one ones-LDW per tile, and the
                # first gram's 128-col LDW pulls ahead under their streaming
                for f0 in range(g_half_main, groups_per_tile, SPAN):
                    gtsum(t, f0)
                for g in range(groups_per_tile):
                    gram(t, g)
            # last tile: chase the quarter DMAs; sum spans are
            # quarter-aligned (issue each span after its last quarter)
            for k in range(N_QUARTERS):
                tensor.wait_ge(qsems[k], 16)
                for f0 in range(g_half_last, groups_per_tile, SPAN):
                    if k * gq <= f0 + SPAN - 1 < (k + 1) * gq:
                        gtsum(last, f0)
                for g in range(k * gq, (k + 1) * gq):
                    final = gram(last, g)
            # the final main-loop instruction carries the completion inc
            final.then_inc(pe_sem, 1)
            assert mm[0] == n_groups and mm[1] == n_sums

    return nc


def _pack_core(pred_c, gt_c, n_tiles=N_TILES, groups_per_tile=GROUPS_PER_TILE):
    """Stage one core's rows as [n_tiles, P, tile_free] fp8 bytes (uint8)."""
    shp = (n_tiles, P, groups_per_tile, R_SUB, C)
    pc = np.asarray(pred_c).reshape(shp)
    gc = np.asarray(gt_c).reshape(shp)
    X = np.empty((n_tiles, P, groups_per_tile, GCOLS), np.uint8)
    # cols are (c, r) pairs, col = c*R_SUB + r -> transpose r and c
    X[..., 0:P] = (pc.transpose(0, 1, 2, 4, 3) != 0).reshape(
        n_tiles, P, groups_per_tile, P) * _F8_ONE
    X[..., P] = _F8_ONE
    X[..., P + 1:GCOLS] = (gc.transpose(0, 1, 2, 4, 3) != 0).reshape(
        n_tiles, P, groups_per_tile, P) * _F8_ONE
    return X.reshape(n_tiles, P, groups_per_tile * GCOLS)


def _unpack_out(o1, o2, o3):
    """Fold one core's raw partials (f64) -> (I, pred_sum, gt_sum), each [C]."""
    diag = o1[np.arange(P), 1 + np.arange(P)]        # I by (c, r)
    inter = diag.reshape(C, R_SUB).sum(axis=1)
    pred_sum = o1[:, 0].reshape(C, R_SUB).sum(axis=1)
    gt_pe = o2.reshape(SPAN, P).sum(axis=0).reshape(C, R_SUB).sum(axis=1)
    gt_dve = o3.reshape(P, -1, C).sum(axis=(0, 1))
    return inter, pred_sum, gt_dve + gt_pe


def _get_nc():
    if "nc" not in _CACHE:
        _CACHE["nc"] = _build_nc()
    return _CACHE["nc"]


def kernel(pred, gt, **run_kwargs):
    global LAST_RUN
    import ml_dtypes
    from concourse.bass_utils import run_bass_kernel_spmd

    pred = np.asarray(pred)
    gt = np.asarray(gt)
    assert pred.shape == (N_ROWS, C) and gt.shape == (N_ROWS, C)

    in_maps = []
    for i in range(N_CORES):
        sl = slice(i * ROWS_PER_CORE, (i + 1) * ROWS_PER_CORE)
        X = _pack_core(pred[sl], gt[sl])
        in_maps.append({"x": X.view(ml_dtypes.float8_e4m3)})

    nc = _get_nc()
    br = run_bass_kernel_spmd(nc, in_maps, core_ids=list(range(N_CORES)),
                              **run_kwargs)
    LAST_RUN = br

    inter = np.zeros(C)
    pred_sum = np.zeros(C)
    gt_sum = np.zeros(C)
    for r in br.results:
        i_, p_, g_ = _unpack_out(r["o1"].astype(np.float64),
                                 r["o2"].astype(np.float64).reshape(-1),
                                 r["o3"].astype(np.float64))
        inter += i_
        pred_sum += p_
        gt_sum += g_
    inter = inter.astype(np.float32)
    pred_sum = pred_sum.astype(np.float32)
    gt_sum = gt_sum.astype(np.float32)

    recalls = (inter + EPS) / (gt_sum + EPS)
    precisions = (inter + EPS) / (pred_sum + EPS)
    return (precisions, recalls, inter, gt_sum, pred_sum)
